# revision 1
# baseline (speedup 1.0000x reference)
"""MultiHeadNonLocalBlock2d on 8 Trainium2 cores.

Sharding: core = (batch b, n-half). Each core computes full 4-head attention
for its 2048 query positions (columns of the [C, N] image), needing full K/V
(all N=4096 positions) but only its slice of Q / residual. No collectives;
host scatters inputs and concatenates outputs.

Layouts (per core):
  qT/kT  [INTER=128(part), n]   -- conv1x1 natural layout (heads stacked, 32 d each)
  v2     [m(part), 4 heads x (32 d | ones | zero-pad to 64)] per m tile
  scoresT[m(part), n]           -- QK^T via 4x-row-tiled PE (K=32 per head)
  exp    bf16, ScalarE, softmax scale folded into ACT affine
  yT     [64(part), n] per head via 64-col-tiled PE; row 32 = sum(exp) (denom)
  z      per-head K=32 row-tiled matmuls (same-row-group accumulation only);
         BN scale/shift and both conv biases folded into host-side vectors
"""

import sys

if '/opt/trn_rl_repo' not in sys.path:
    sys.path.insert(0, '/opt/trn_rl_repo')

from contextlib import ExitStack

import ml_dtypes
import numpy as np

import concourse.bass as bass
import concourse.mybir as mybir
import concourse.tile as tile
from concourse import bacc, bass_utils
from concourse.bass import ts

F32 = mybir.dt.float32
BF16 = mybir.dt.bfloat16
BF = ml_dtypes.bfloat16

B, C, H, W = 4, 256, 64, 64
INTER, HEADS = 128, 4
D = INTER // HEADS          # 32
N = H * W                   # 4096
EPS = 1e-5
NCORE = 8
NH = N // 2                 # 2048 per-core query range
CH = 512                    # n-chunk
NCH = NH // CH              # 4 chunks
MT = N // 128               # 32 m-tiles
ALPHA = float(D) ** -0.5

Exp = mybir.ActivationFunctionType.Exp
MULT = mybir.AluOpType.mult
ADD = mybir.AluOpType.add


def build():
    nc = bacc.Bacc("TRN2", target_bir_lowering=False, debug=False)

    # ---- DRAM I/O ----
    xb = [nc.dram_tensor(f"xb{k}", [128, N], BF16, kind="ExternalInput")
          for k in range(2)]                       # full x[b] (bf16), C k-tiles
    xq = [nc.dram_tensor(f"xq{k}", [128, NH], F32, kind="ExternalInput")
          for k in range(2)]                       # residual slice (f32)
    # all weights packed in one tensor: wq0|wq1|wk0|wk1|wg0|wg1|wzA(512)
    # wzA: w_z.T rows rearranged so rg0 holds heads 0/2, rg2 holds heads 1/3
    wp_d = nc.dram_tensor("wpack", [128, 1280], BF16, kind="ExternalInput")
    # bq | bk | bnsc(2)
    bp_d = nc.dram_tensor("bpack", [128, 4], F32, kind="ExternalInput")
    # rows 32/96 hold head-block indicators for the recip broadcast matmul
    ind_d = nc.dram_tensor("ind2", [128, 128], BF16, kind="ExternalInput")
    out_d = nc.dram_tensor("out", [2, 128, NH], F32, kind="ExternalOutput")

    with tile.TileContext(nc) as tc, ExitStack() as ctx:
        const = ctx.enter_context(tc.tile_pool(name="const", bufs=1))
        sb = ctx.enter_context(tc.tile_pool(name="sb", bufs=1))
        sbm = ctx.enter_context(tc.tile_pool(name="sbm", bufs=2))
        expool = ctx.enter_context(tc.tile_pool(name="expool", bufs=4))

        def load(dram, shape, dtype):
            t = const.tile(shape, dtype, tag=dram.name, name=dram.name + "_t")
            nc.sync.dma_start(t[:], dram.ap())
            return t

        # DMA issue order = dependency order, minimizing HWDGE slots on the
        # critical path: packed weights, then q inputs, then x (2 pieces each
        # so the first kT/v2 projections start early). The f32 residual and
        # ind (epilogue-only) ride the ACT HWDGE queue instead.
        # wq slice first -- it alone gates proj_q(0); the rest of the
        # weights follow after the q inputs
        wp_t = const.tile([128, 1280], BF16, tag="wpack", name="wpack_t")
        nc.sync.dma_start(wp_t[:, 0:768], wp_d.ap()[:, 0:768])
        wq_t = [wp_t[:, ts(k, 128)] for k in range(2)]
        wk_t = [wp_t[:, ts(2 + k, 128)] for k in range(2)]
        wg_t = [wp_t[:, ts(4 + k, 128)] for k in range(2)]
        wza_t = wp_t[:, 768:1280]

        # transfer order = need order: q-proj chunk 0 inputs, then x (halves
        # interleaved across the two C k-tiles so the first kT/v2 projections
        # start after half the x traffic), then the rest of xqb, then
        # epilogue-only inputs -- the DMA pipeline drains in arrival order
        # xb arrives with this core's own query half FIRST (host reorders;
        # attention is invariant to permuting key/value positions), so the
        # q-projection reads xb directly -- no separate xqb traffic.
        xb_t = [const.tile([128, N], BF16, tag=xb[k].name, name=xb[k].name + "_t")
                for k in range(2)]
        for k in range(2):
            nc.sync.dma_start(xb_t[k][:, 0:CH], xb[k].ap()[:, 0:CH])
        bp_t = load(bp_d, [128, 4], F32)
        bq_t, bk_t, bnsc_t = bp_t[:, 0:1], bp_t[:, 1:2], bp_t[:, 2:4]
        nc.sync.dma_start(wp_t[:, 768:1280], wp_d.ap()[:, 768:1280])
        for c0, c1 in ((CH, 1024), (1024, 2048), (2048, 3072), (3072, N)):
            for k in range(2):
                nc.sync.dma_start(xb_t[k][:, c0:c1], xb[k].ap()[:, c0:c1])
        ind_t = load(ind_d, [128, 128], BF16)
        xq_t = [load(xq[k], [128, NH], F32) for k in range(2)]

        kT = sb.tile([128, N], BF16, tag="kT", name="kT")
        qT = sb.tile([128, NH], BF16, tag="qT", name="qT")
        # per (m-tile, head): [32 v cols | ones col | 31 zero cols] -- M=64
        # AV matmuls then cover all psum partitions (no uninit rows)
        v2 = sb.tile([128, MT * 256], BF16, tag="v2", name="v2")
        nc.gpsimd.memset(v2[:], 0.0)
        v2ones = v2[:].rearrange("p (c w) -> p c w", w=64)[:, :, 32:33]
        nc.gpsimd.memset(v2ones, 1.0)
        rcp = sb.tile([128, 2 * CH], BF16, tag="rcp", name="rcp")
        nc.gpsimd.memset(rcp[:], 0.0)  # only rows 32/96 ever written

        with tc.tile_pool(name="scps", bufs=2, space="PSUM") as scps, \
             tc.tile_pool(name="ytps", bufs=2, space="PSUM") as ytps, \
             tc.tile_pool(name="mps", bufs=2, space="PSUM") as mps:

            # ---- projection emitters (interleaved into chunk-0 schedule) ----
            def proj_q(j):
                ps = mps.tile([128, CH], F32, tag="m", name="qps")
                nc.tensor.matmul(ps[:], wq_t[0][:], xb_t[0][:, ts(j, CH)],
                                 start=True, stop=False)
                nc.tensor.matmul(ps[:], wq_t[1][:], xb_t[1][:, ts(j, CH)],
                                 start=False, stop=True)
                nc.vector.tensor_scalar_add(qT[:, ts(j, CH)], ps[:], bq_t[:, 0:1])

            def proj_k(j):
                ps = mps.tile([128, CH], F32, tag="m", name="kps")
                nc.tensor.matmul(ps[:], wk_t[0][:], xb_t[0][:, ts(j, CH)],
                                 start=True, stop=False)
                nc.tensor.matmul(ps[:], wk_t[1][:], xb_t[1][:, ts(j, CH)],
                                 start=False, stop=True)
                nc.vector.tensor_scalar_add(kT[:, ts(j, CH)], ps[:], bk_t[:, 0:1])

            def proj_v(mt):
                ps = mps.tile([128, 128], F32, tag="m", name="vps")
                nc.tensor.matmul(ps[:], xb_t[0][:, ts(mt, 128)], wg_t[0][:],
                                 start=True, stop=False)
                nc.tensor.matmul(ps[:], xb_t[1][:, ts(mt, 128)], wg_t[1][:],
                                 start=False, stop=True)
                seg = v2[:, 256 * mt:256 * (mt + 1)]
                dst = seg.rearrange("p (h w) -> p h w", h=4)[:, :, 0:32]
                src = ps[:].rearrange("p (h w) -> p h w", h=4)
                nc.vector.tensor_copy(dst, src)

            def proj_k_cols(c0, w):
                ps = mps.tile([128, CH], F32, tag="m", name="kps0")
                nc.tensor.matmul(ps[:, 0:w], wk_t[0][:], xb_t[0][:, c0:c0 + w],
                                 start=True, stop=False)
                nc.tensor.matmul(ps[:, 0:w], wk_t[1][:], xb_t[1][:, c0:c0 + w],
                                 start=False, stop=True)
                nc.vector.tensor_scalar_add(kT[:, c0:c0 + w], ps[:, 0:w],
                                            bk_t[:, 0:1])

            # only qT chunk 0 and kT cols 0:128 gate the first step;
            # everything else streams in during chunk 0
            proj_q(0)
            proj_k_cols(0, 128)
            proj_k_cols(128, 384)

            def norm_pair(pr, ysbs):
                # broadcast this pair's denominator reciprocals (rcp rows
                # 32/96) to its head-block partitions via an indicator matmul,
                # then normalize
                bc = mps.tile([128, CH], F32, tag="m", name="bc")
                nc.tensor.matmul(bc[:], ind_t[:], rcp[:, ts(pr, CH)],
                                 start=True, stop=True)
                y_n = sbm.tile([128, CH], BF16, tag="yn", name="y_n")
                nc.vector.tensor_tensor(y_n[:], ysbs[pr][:], bc[:], op=MULT)
                return y_n

            def z_mm(zps, mtz, g, pr, y_n, start, stop):
                # heads 0/2 accumulate in row-group 0, heads 1/3 in
                # row-group 2 (cross-group accumulation is a HW fault);
                # the two groups are summed on DVE in the BN combine
                nc.tensor.matmul(
                    zps[:],
                    wza_t[64 * g:64 * g + 32,
                          256 * pr + 128 * mtz:256 * pr + 128 * mtz + 128],
                    y_n[64 * g:64 * g + 32, :],
                    start=start, stop=stop,
                    tile_position=(64 * g, 0))

            def z_out(ch, mtz, zab):
                ot = sbm.tile([128, CH], F32, tag="ot", name="ot")
                # out = (za + zb) * bn_scale + (x + bn_shift)
                nc.vector.scalar_tensor_tensor(
                    ot[:], zab[0][:], bnsc_t[:, mtz:mtz + 1],
                    xq_t[mtz][:, ts(ch, CH)], op0=MULT, op1=ADD)
                nc.vector.scalar_tensor_tensor(
                    ot[:], zab[1][:], bnsc_t[:, mtz:mtz + 1], ot[:],
                    op0=MULT, op1=ADD)
                nc.sync.dma_start(out_d.ap()[mtz, :, ts(ch, CH)], ot[:])

            def epilogue(ch, ysbs):
                yns = [norm_pair(pr, ysbs) for pr in range(2)]
                for mtz in range(2):
                    zab = []
                    for g in range(2):
                        zps = mps.tile([128, CH], F32, tag="m", name="zps")
                        for pr in range(2):
                            z_mm(zps, mtz, g, pr, yns[pr], pr == 0, pr == 1)
                        zab.append(zps)
                    z_out(ch, mtz, zab)

            # ---- global step stream across all chunks ----
            # AV trails QK/exp by 2 steps and flows across chunk boundaries so
            # the PE never serializes a chunk transition; each chunk's
            # normalize/z epilogue is deferred until the next chunk is rolling.
            state = {"pending": None}
            queue = []

            def do_av(mt, pr, ex, ys):
                y = ys[pr]
                for i in range(2):
                    h = 2 * pr + i
                    nc.tensor.matmul(
                        y[64 * i:64 * i + 64, :],
                        v2[:, 256 * mt + 64 * h:256 * mt + 64 * h + 64],
                        ex[:, ts(i, CH)],
                        start=(mt == 0), stop=(mt == MT - 1),
                        tile_position=(0, 64 * i),
                        skip_group_check=True,
                    )

            def post(pr, ys, st):
                # denominator recips straight from PSUM (unblocks the
                # broadcast matmul), then copy yT out of psum (frees the bank)
                with nc.allow_low_precision("softmax denominators in bf16"):
                    nc.vector.reciprocal(rcp[32:33, ts(pr, CH)], ys[pr][32:33, :])
                    nc.vector.reciprocal(rcp[96:97, ts(pr, CH)], ys[pr][96:97, :])
                ysb = sbm.tile([128, CH], F32, tag="ysb", name="ysb")
                nc.vector.tensor_copy(ysb[:], ys[pr][:])
                st["ysbs"][pr] = ysb

            def pop_av():
                mt, pr, ex, ys, st = queue.pop(0)
                do_av(mt, pr, ex, ys)
                if mt == MT - 1:
                    post(pr, ys, st)
                    if pr == 1:
                        state["pending"] = (lambda st=st:
                                            epilogue(st["ch"], st["ysbs"]))

            for ch in range(NCH):
                ys = [ytps.tile([128, CH], F32, tag="yt", name=f"yt{ch}_{p}")
                      for p in range(2)]
                st = {"ch": ch, "ysbs": [None, None]}
                for mt in range(MT):
                    if ch == 0:
                        if mt % 4 == 1 and mt < MT - 3:
                            proj_k(mt // 4 + 1)
                        if mt in (2, 4, 6):
                            proj_q(mt // 2)
                        proj_v(mt)
                    for pr in range(2):
                        step = 2 * mt + pr
                        s_ps = scps.tile([128, 2 * CH], F32, tag="s", name="sps")
                        for i in range(2):
                            h = 2 * pr + i
                            nc.tensor.matmul(
                                s_ps[:, ts(i, CH)],
                                kT[32 * h:32 * h + 32, ts(mt, 128)],
                                qT[32 * h:32 * h + 32, ts(ch, CH)],
                                start=True, stop=True,
                                tile_position=(32 * h, 0),
                            )
                        ex = expool.tile([128, 2 * CH], BF16, tag="e", name="ex")
                        nc.scalar.activation(ex[:], s_ps[:], Exp, scale=ALPHA)
                        queue.append((mt, pr, ex, ys, st))
                        if len(queue) > 2:
                            pop_av()
                        if state["pending"] is not None and step == 12:
                            state["pending"]()
                            state["pending"] = None
            # drain: pair 0's normalize and z-partials overlap the last exp
            # and pair 1's AV; only pair 1's half-chain trails the last exp
            st = queue[0][4]
            pop_av()                       # AV(31,0) + post(0)
            mt1, pr1, ex1, ys1, _ = queue.pop(0)
            do_av(mt1, pr1, ex1, ys1)      # AV(31,1) -- ahead of pair-0 PE work
            # ACT (idle post-stream) copies yT out first; the recips then
            # read the SBUF copy -- same-bank psum readers would serialize
            ysb1 = sbm.tile([128, CH], F32, tag="ysb", name="ysb1")
            nc.scalar.copy(ysb1[:], ys1[1][:])
            with nc.allow_low_precision("softmax denominators in bf16"):
                nc.vector.reciprocal(rcp[32:33, ts(1, CH)], ysb1[32:33, :])
                nc.vector.reciprocal(rcp[96:97, ts(1, CH)], ysb1[96:97, :])
            st["ysbs"][1] = ysb1
            yn0 = norm_pair(0, st["ysbs"])
            zuf = [scps.tile([128, CH], F32, tag="s", name="zf"),
                   ytps.tile([128, CH], F32, tag="yt", name="zf2")]
            zparts = [[zuf[0],
                       mps.tile([128, CH], F32, tag="m", name="zf3")],
                      [zuf[1],
                       scps.tile([128, CH], F32, tag="s", name="zf4")]]
            for mtz in range(2):
                for g in range(2):
                    z_mm(zparts[mtz][g], mtz, g, 0, yn0, True, False)
            yn1 = norm_pair(1, st["ysbs"])
            for mtz in range(2):
                for g in range(2):
                    z_mm(zparts[mtz][g], mtz, g, 1, yn1, False, True)
                z_out(NCH - 1, mtz, zparts[mtz])

    nc.compile()
    return nc


_NC = None


def _get_nc():
    global _NC
    if _NC is None:
        _NC = build()
    return _NC


def _in_maps(x, w_theta, b_theta, w_phi, b_phi, w_g, b_g, w_z, b_z,
             bn_gamma, bn_beta, bn_mean, bn_var):
    xr = np.ascontiguousarray(np.asarray(x, np.float32).reshape(B, C, N))
    wqT = np.asarray(w_theta, np.float32).T.astype(BF)   # [256, 128]
    wkT = np.asarray(w_phi, np.float32).T.astype(BF)
    wgT = np.asarray(w_g, np.float32).T.astype(BF)
    wzT32 = np.asarray(w_z, np.float32).T.reshape(4, 32, 256)  # per head
    inv = np.asarray(bn_gamma, np.float32) / np.sqrt(np.asarray(bn_var, np.float32) + EPS)
    shift = ((np.asarray(w_z, np.float32) @ np.asarray(b_g, np.float32)
              + np.asarray(b_z, np.float32)) * inv
             + np.asarray(bn_beta, np.float32)
             - np.asarray(bn_mean, np.float32) * inv)
    bnsc = np.ascontiguousarray(inv.reshape(2, 128).T)   # [128, 2] col=mtile
    ind2 = np.zeros((128, 128), BF)
    ind2[32, 0:32] = 1.0
    ind2[96, 64:96] = 1.0

    wzA = np.zeros((128, 512), np.float32)
    for g in range(2):          # row group; heads g, 2+g
        for pr in range(2):
            h = 2 * pr + g
            wzA[64 * g:64 * g + 32, 256 * pr:256 * pr + 256] = wzT32[h]
    wpack = np.concatenate([wqT[:128], wqT[128:], wkT[:128], wkT[128:],
                            wgT[:128], wgT[128:], wzA.astype(BF)], axis=1)
    bpack = np.stack([np.asarray(b_theta, np.float32),
                      np.asarray(b_phi, np.float32),
                      bnsc[:, 0], bnsc[:, 1]], axis=1)
    shared = {
        "wpack": np.ascontiguousarray(wpack),
        "bpack": np.ascontiguousarray(bpack),
        "ind2": ind2,
    }
    maps = []
    for core in range(NCORE):
        b, half = divmod(core, 2)
        n0 = half * NH
        n1 = NH - n0                        # the other half
        xre = np.concatenate([xr[b][:, n0:n0 + NH], xr[b][:, n1:n1 + NH]],
                             axis=1)         # own query half first
        xbf = xre.astype(BF)
        xqres = xre[:, 0:NH] + shift[:, None]  # residual w/ bn shift folded in
        m = dict(shared)
        m["xb0"] = np.ascontiguousarray(xbf[:128])
        m["xb1"] = np.ascontiguousarray(xbf[128:])
        m["xq0"] = np.ascontiguousarray(xqres[:128].astype(np.float32))
        m["xq1"] = np.ascontiguousarray(xqres[128:].astype(np.float32))
        maps.append(m)
    return maps


def kernel(**inputs):
    nc = _get_nc()
    maps = _in_maps(**inputs)
    res = bass_utils.run_bass_kernel_spmd(nc, maps, core_ids=list(range(NCORE)))
    out = np.empty((B, C, N), np.float32)
    for core in range(NCORE):
        b, half = divmod(core, 2)
        n0 = half * NH
        out[b][:, n0:n0 + NH] = res.results[core]["out"].reshape(C, NH)
    return out.reshape(B, C, H, W)



# revision 5
# speedup vs baseline: 1.4354x; 1.4354x over previous
"""MultiHeadNonLocalBlock2d on 8 Trainium2 cores.

Sharding: core = (batch b, n-half): 2048 queries x 4096 keys x 4 heads.

Per-core pipeline (cost-model-shaped):
  proj    fp8 DoubleRow (K=256 as 2 planes) -> psum f32
  q/k     ACT Identity+bias fold -> fp8 plane-0 of qT8/kT8 (plane-1 zeros)
  QK      fp8 DoubleRow, zero plane-1 pads K=32->64; out scoresT [keys, q]
  exp     split ACT (exact Exp -> bf16) / DVE (int16 Schraudolph, bits
          reinterpreted as bf16) -- the two engines are the wall
  AV      exp blocks as PE *weights* (ldweights is free), v [keys,33]
          moving incl. ones col -> psum chains [q, 32d | denom]
  norm    DVE reciprocal + stride-0-broadcast multiply -> yn bf16
  yT      PE transposes (col-banded) + ACT copy -> [hd, q]
  z       single K=128 matmul per c-half (bn inv folded into w_z)
  out     DVE z + residual(x + bn shift) -> DMA
"""

import sys

if '/opt/trn_rl_repo' not in sys.path:
    sys.path.insert(0, '/opt/trn_rl_repo')

from contextlib import ExitStack

import ml_dtypes
import numpy as np

import concourse.bass as bass
import concourse.mybir as mybir
import concourse.tile as tile
from concourse import bacc, bass_utils

F32 = mybir.dt.float32
BF16 = mybir.dt.bfloat16
I16 = mybir.dt.int16
FP8 = mybir.dt.float8e4
BF = ml_dtypes.bfloat16
F8 = ml_dtypes.float8_e4m3

B, C, H, W = 4, 256, 64, 64
INTER, HEADS = 128, 4
D = INTER // HEADS          # 32
N = H * W                   # 4096
EPS = 1e-5
NCORE = 8
NH = N // 2                 # queries per core
CH = 512                    # query chunk
NCH = NH // CH              # 4
MT = N // 128               # 32 key tiles
ALPHA = float(D) ** -0.5

MULT = mybir.AluOpType.mult
ADD = mybir.AluOpType.add
Exp = mybir.ActivationFunctionType.Exp
Ident = mybir.ActivationFunctionType.Identity
DR = mybir.MatmulPerfMode.DoubleRow

LOG2E = 1.4426950408889634
A16 = 128.0 * LOG2E
B16 = 127.0 * 128.0 - 4.0

SPLIT_A = 0.54              # fraction of exp pair-tiles on ACT
AVLAG = 3                   # AV trails exp by this many pair-steps


def build():
    nc = bacc.Bacc("TRN2", target_bir_lowering=False, debug=False)

    x8_d = nc.dram_tensor("x8", [128, 2 * N], FP8, kind="ExternalInput")
    xq_d = nc.dram_tensor("xq", [128, 2 * NH], F32, kind="ExternalInput")
    wp8_d = nc.dram_tensor("wp8", [128, 768], FP8, kind="ExternalInput")
    wz_d = nc.dram_tensor("wz", [128, 256], BF16, kind="ExternalInput")
    idn_d = nc.dram_tensor("idn", [128, 128], BF16, kind="ExternalInput")
    bqk_d = nc.dram_tensor("bqk", [128, 2], F32, kind="ExternalInput")
    out_d = nc.dram_tensor("out", [2, 128, NH], F32, kind="ExternalOutput")

    with tile.TileContext(nc) as tc, ExitStack() as ctx:
        const = ctx.enter_context(tc.tile_pool(name="const", bufs=1))
        sb = ctx.enter_context(tc.tile_pool(name="sb", bufs=1))
        exa = ctx.enter_context(tc.tile_pool(name="exa", bufs=3))
        exd = ctx.enter_context(tc.tile_pool(name="exd", bufs=3))
        ynp = ctx.enter_context(tc.tile_pool(name="ynp", bufs=2))
        ytp = ctx.enter_context(tc.tile_pool(name="ytp", bufs=2))
        rcpp = ctx.enter_context(tc.tile_pool(name="rcpp", bufs=2))
        otp = ctx.enter_context(tc.tile_pool(name="otp", bufs=2))

        # ---- DMA preamble (issue order = need order) ----
        wp8_t = const.tile([128, 768], FP8, tag="wp8", name="wp8_t")
        nc.sync.dma_start(wp8_t[:], wp8_d.ap())
        bqk_t = const.tile([128, 2], F32, tag="bqk", name="bqk_t")
        nc.sync.dma_start(bqk_t[:], bqk_d.ap())
        # x8 is plane-major [2, N]; load both planes of each 512-col chunk
        # together so projection chunk j has its full K=256 early
        x8_t = const.tile([128, 2 * N], FP8, tag="x8", name="x8_t")
        for c0 in range(0, N, 512):
            nc.sync.dma_start(x8_t[:, c0:c0 + 512], x8_d.ap()[:, c0:c0 + 512])
            nc.sync.dma_start(x8_t[:, N + c0:N + c0 + 512],
                              x8_d.ap()[:, N + c0:N + c0 + 512])
        wz_t = const.tile([128, 256], BF16, tag="wz", name="wz_t")
        nc.sync.dma_start(wz_t[:], wz_d.ap())
        idn_t = const.tile([128, 128], BF16, tag="idn", name="idn_t")
        nc.sync.dma_start(idn_t[:], idn_d.ap())
        xq_t = const.tile([128, 2 * NH], F32, tag="xq", name="xq_t")
        for c0 in range(0, 2 * NH, 2048):
            nc.sync.dma_start(xq_t[:, c0:c0 + 2048], xq_d.ap()[:, c0:c0 + 2048])

        x8v = x8_t[:].rearrange("p (two n) -> p two n", two=2)       # [128,2,N]
        wq8 = wp8_t[:, 0:256].rearrange("p (two m) -> p two m", two=2)
        wk8 = wp8_t[:, 256:512].rearrange("p (two m) -> p two m", two=2)
        wg8 = wp8_t[:, 512:768].rearrange("p (two m) -> p two m", two=2)

        # ---- persistent SBUF ----
        qT8 = sb.tile([128, 2 * NH], FP8, tag="qT8", name="qT8")     # (2,NH)
        kT8 = sb.tile([128, 2 * N], FP8, tag="kT8", name="kT8")      # (2,N)
        v2 = sb.tile([128, MT * 132], BF16, tag="v2", name="v2")

        # zero planes / ones col (Pool engine, ordered by tile deps)
        nc.gpsimd.memset(kT8[:, N:N + 512], 0.0)          # covers mt 0..3
        nc.gpsimd.memset(qT8[:, NH:NH + CH], 0.0)         # ch0 plane-1
        v2ones = v2[:].rearrange("p (c w) -> p c w", w=33)[:, :, 32:33]
        nc.gpsimd.memset(v2ones, 1.0)
        nc.gpsimd.memset(kT8[:, N + 512:2 * N], 0.0)
        nc.gpsimd.memset(qT8[:, NH + CH:2 * NH], 0.0)

        with tc.tile_pool(name="sc", bufs=3, space="PSUM") as sc, \
             tc.tile_pool(name="av", bufs=1, space="PSUM") as av:

            # ---- projection emitters ----
            def proj_q(j):
                ps = sc.tile([128, 1024], F32, tag="s", name="qps")
                nc.tensor.matmul(ps[:, 0:CH], wq8,
                                 x8v[:, :, j * CH:(j + 1) * CH],
                                 start=True, stop=True, perf_mode=DR)
                with nc.allow_low_precision("q fold fp8"):
                    nc.scalar.activation(qT8[:, j * CH:(j + 1) * CH],
                                         ps[:, 0:CH], Ident,
                                         bias=bqk_t[:, 0:1])

            def proj_k(j):
                ps = sc.tile([128, 1024], F32, tag="s", name="kps")
                nc.tensor.matmul(ps[:, 0:CH], wk8,
                                 x8v[:, :, j * CH:(j + 1) * CH],
                                 start=True, stop=True, perf_mode=DR)
                with nc.allow_low_precision("k fold fp8"):
                    nc.scalar.activation(kT8[:, j * CH:(j + 1) * CH],
                                         ps[:, 0:CH], Ident,
                                         bias=bqk_t[:, 1:2])

            def proj_v(m2):
                # two key tiles 2*m2, 2*m2+1 -> v2 cols
                ps = sc.tile([128, 1024], F32, tag="s", name="vps")
                for i in range(2):
                    m = 2 * m2 + i
                    nc.tensor.matmul(ps[:, i * 128:i * 128 + 128],
                                     x8v[:, :, m * 128:(m + 1) * 128], wg8,
                                     start=True, stop=True, perf_mode=DR)
                dst = v2[:, m2 * 264:(m2 + 1) * 264] \
                    .rearrange("p (c w) -> p c w", w=33)[:, :, 0:32]
                src = ps[:, 0:256].rearrange("p (c w) -> p c w", w=32)
                with nc.allow_low_precision("v2 bf16"):
                    nc.scalar.copy(dst, src)

            qv = qT8[:].rearrange("p (two n) -> p two n", two=2)
            kv = kT8[:].rearrange("p (two n) -> p two n", two=2)

            # ---- AV + epilogue ----
            st_chains = {}

            def do_av(ch, mt, pr, ex, exbf):
                if mt == 0 and pr == 0:
                    st_chains[ch] = av.tile([128, 1024], F32, tag="av",
                                            name=f"chains{ch}")
                chains = st_chains[ch]
                for i in range(2):
                    h = 2 * pr + i
                    for qt in range(4):
                        c = 4 * h + qt
                        nc.tensor.matmul(
                            chains[:, 64 * c:64 * c + 33],
                            exbf[:, i * CH + qt * 128:i * CH + qt * 128 + 128],
                            v2[:, mt * 132 + h * 33:mt * 132 + h * 33 + 33],
                            start=(mt == 0 and c % 8 == 0), stop=(mt == MT - 1),
                            skip_group_check=True)

            def epilogue(ch):
                chains = st_chains.pop(ch)
                cv = chains[:].rearrange("p (c w) -> p c w", w=64)
                rcp = rcpp.tile([128, 16], F32, tag="r", name="rcp")
                for b_ in range(2):
                    nc.vector.reciprocal(
                        rcp[:, 8 * b_:8 * b_ + 8]
                        .rearrange("p (c u) -> p c u", u=1),
                        cv[:, 8 * b_:8 * b_ + 8, 32:33])
                yn = ynp.tile([128, 512], BF16, tag="yn", name="yn")
                ynv = yn[:].rearrange("p (q w) -> p q w", w=128)
                with nc.allow_low_precision("normalize bf16"):
                    for qt in range(4):
                        nc.vector.tensor_tensor(
                            ynv[:, qt, :].rearrange("p (c w) -> p c w", w=32),
                            cv[:, qt::4, 0:32],
                            rcp[:, qt:16:4].rearrange("p (c u) -> p c u", u=1)
                            .broadcast_to([128, 4, 32]),
                            op=MULT)
                epi = sc.tile([128, 1024], F32, tag="s", name="epi")
                ytv = epi[:, 0:256].bitcast(BF16)       # [128, 512] bf16
                for qt in range(4):
                    for h in range(4):
                        nc.tensor.matmul(
                            ytv[32 * h:32 * h + 32, 128 * qt:128 * qt + 128],
                            yn[:, 128 * qt + 32 * h:128 * qt + 32 * h + 32],
                            idn_t[:], is_transpose=True,
                            start=False, stop=True, skip_group_check=True,
                            tile_position=(0, 32 * h))
                yT = ytp.tile([128, 512], BF16, tag="yT", name="yT")
                nc.scalar.copy(yT[:], ytv[:])
                zt = av.tile([128, 1024], F32, tag="av", name=f"zt{ch}")
                nc.tensor.matmul(epi[:, 512:1024], wz_t[:, 0:128], yT[:],
                                 start=True, stop=True)
                nc.tensor.matmul(zt[:, 0:512], wz_t[:, 128:256], yT[:],
                                 start=True, stop=True)
                for mtz in range(2):
                    zsrc = epi[:, 512:1024] if mtz == 0 else zt[:, 0:512]
                    o = otp.tile([128, 512], F32, tag="o", name="ot")
                    nc.vector.tensor_tensor(
                        o[:], zsrc, xq_t[:, mtz * NH + ch * CH:
                                         mtz * NH + ch * CH + CH], op=ADD)
                    nc.sync.dma_start(out_d.ap()[mtz, :, ch * CH:(ch + 1) * CH],
                                      o[:])

            # ---- main stream ----
            proj_q(0)
            proj_k(0)
            proj_v(0)

            queue = []
            acc = [0.0]

            def pick_act():
                acc[0] += SPLIT_A
                if acc[0] >= 1.0:
                    acc[0] -= 1.0
                    return True
                return False

            def pop_av():
                ch, mt, pr, ex, exbf = queue.pop(0)
                do_av(ch, mt, pr, ex, exbf)
                if mt == MT - 1 and pr == 1:
                    epilogue(ch)

            for ch in range(NCH):
                for mt in range(MT):
                    if ch == 0:
                        if mt % 4 == 1 and mt < 29:
                            proj_k(mt // 4 + 1)
                        if mt in (3, 11, 19) and mt // 8 + 1 < NCH:
                            proj_q(mt // 8 + 1)
                        if mt % 2 == 0 and mt >= 2:
                            proj_v(mt // 2)
                        if mt == 30:
                            proj_v(15)
                    for pr in range(2):
                        s = sc.tile([128, 1024], F32, tag="s", name="sps")
                        for i in range(2):
                            h = 2 * pr + i
                            nc.tensor.matmul(
                                s[:, i * CH:(i + 1) * CH],
                                kv[32 * h:32 * h + 32, :, mt * 128:(mt + 1) * 128],
                                qv[32 * h:32 * h + 32, :, ch * CH:(ch + 1) * CH],
                                start=True, stop=True, perf_mode=DR,
                                tile_position=(32 * h, 0))
                        if pick_act():
                            ex = exa.tile([128, 1024], BF16, tag="e", name="exa")
                            nc.scalar.activation(ex[:], s[:], Exp)
                            exbf = ex[:]
                        else:
                            ex = exd.tile([128, 1024], I16, tag="e", name="exd")
                            with nc.allow_low_precision("schraudolph exp"):
                                nc.vector.tensor_scalar(ex[:], s[:], A16, B16,
                                                        op0=MULT, op1=ADD)
                            exbf = ex[:].bitcast(BF16)
                        queue.append((ch, mt, pr, ex, exbf))
                        if len(queue) > AVLAG:
                            pop_av()
            while queue:
                pop_av()

    nc.compile()
    return nc


_NC = None


def _get_nc():
    global _NC
    if _NC is None:
        _NC = build()
    return _NC


def _in_maps(x, w_theta, b_theta, w_phi, b_phi, w_g, b_g, w_z, b_z,
             bn_gamma, bn_beta, bn_mean, bn_var):
    sqa = np.float32(np.sqrt(ALPHA))
    inv = (np.asarray(bn_gamma, np.float32)
           / np.sqrt(np.asarray(bn_var, np.float32) + EPS))
    shift = ((np.asarray(w_z, np.float32) @ np.asarray(b_g, np.float32)
              + np.asarray(b_z, np.float32)) * inv
             + np.asarray(bn_beta, np.float32)
             - np.asarray(bn_mean, np.float32) * inv)

    def pack_w(w):  # [INTER, C] -> [128, 256] lhsT planes, fp8
        wT = np.asarray(w, np.float32).T           # [C, INTER]
        return np.concatenate([wT[:128], wT[128:]], axis=1)

    wq = pack_w(np.asarray(w_theta, np.float32) * sqa)
    wk = pack_w(np.asarray(w_phi, np.float32) * sqa)
    wg = pack_w(np.asarray(w_g, np.float32))
    wp8 = np.ascontiguousarray(
        np.concatenate([wq, wk, wg], axis=1)).astype(F8)
    wzs = np.ascontiguousarray(
        (np.asarray(w_z, np.float32) * inv[:, None]).T.astype(BF))  # [128,256]
    bqk = np.ascontiguousarray(np.stack(
        [np.asarray(b_theta, np.float32) * sqa,
         np.asarray(b_phi, np.float32) * sqa], axis=1))
    idn = np.eye(128, dtype=np.float32).astype(BF)

    xr = np.asarray(x, np.float32).reshape(B, C, N)
    shared = {"wp8": wp8, "wz": wzs, "bqk": bqk, "idn": idn}
    maps = []
    for core in range(NCORE):
        b_, half = divmod(core, 2)
        n0 = half * NH
        n1 = NH - n0
        xre = np.concatenate([xr[b_][:, n0:n0 + NH], xr[b_][:, n1:n1 + NH]],
                             axis=1)            # own query half first
        xs = xre.astype(F8)
        x8 = np.empty((128, 2 * N), F8)
        x8[:, 0:N] = xs[:128]
        x8[:, N:2 * N] = xs[128:]
        xqr = xre[:, 0:NH] + shift[:, None]
        xqp = np.concatenate([xqr[:128], xqr[128:]], axis=1).astype(np.float32)
        m = dict(shared)
        m["x8"] = np.ascontiguousarray(x8)
        m["xq"] = np.ascontiguousarray(xqp)
        maps.append(m)
    return maps


def kernel(**inputs):
    nc = _get_nc()
    maps = _in_maps(**inputs)
    res = bass_utils.run_bass_kernel_spmd(nc, maps, core_ids=list(range(NCORE)))
    out = np.empty((B, C, N), np.float32)
    for core in range(NCORE):
        b_, half = divmod(core, 2)
        n0 = half * NH
        out[b_][:, n0:n0 + NH] = res.results[core]["out"].reshape(C, NH)
    return out.reshape(B, C, H, W)


# revision 9
# speedup vs baseline: 1.4711x; 1.0248x over previous
"""MultiHeadNonLocalBlock2d on 8 Trainium2 cores.

Sharding: core = (batch b, n-half): 2048 queries x 4096 keys x 4 heads.

Per-core pipeline (cost-model-shaped):
  proj    fp8 DoubleRow (K=256 as 2 planes) -> psum f32
  q/k     ACT Identity+bias fold -> fp8 plane-0 of qT8/kT8 (plane-1 zeros)
  QK      fp8 DoubleRow, zero plane-1 pads K=32->64; out scoresT [keys, q]
  exp     split ACT (exact Exp -> bf16) / DVE (int16 Schraudolph, bits
          reinterpreted as bf16) -- the two engines are the wall
  AV      exp blocks as PE *weights* (ldweights is free), v [keys,33]
          moving incl. ones col -> psum chains [q, 32d | denom]
  norm    DVE reciprocal + stride-0-broadcast multiply -> yn bf16
  yT      PE transposes (col-banded) + ACT copy -> [hd, q]
  z       single K=128 matmul per c-half (bn inv folded into w_z)
  out     DVE z + residual(x + bn shift) -> DMA
"""

import sys

if '/opt/trn_rl_repo' not in sys.path:
    sys.path.insert(0, '/opt/trn_rl_repo')

from contextlib import ExitStack

import ml_dtypes
import numpy as np

import concourse.bass as bass
import concourse.mybir as mybir
import concourse.tile as tile
from concourse import bacc, bass_utils

F32 = mybir.dt.float32
BF16 = mybir.dt.bfloat16
I16 = mybir.dt.int16
FP8 = mybir.dt.float8e4
BF = ml_dtypes.bfloat16
F8 = ml_dtypes.float8_e4m3

B, C, H, W = 4, 256, 64, 64
INTER, HEADS = 128, 4
D = INTER // HEADS          # 32
N = H * W                   # 4096
EPS = 1e-5
NCORE = 8
NH = N // 2                 # queries per core
CH = 512                    # query chunk
NCH = NH // CH              # 4
MT = N // 128               # 32 key tiles
ALPHA = float(D) ** -0.5

MULT = mybir.AluOpType.mult
ADD = mybir.AluOpType.add
Exp = mybir.ActivationFunctionType.Exp
Ident = mybir.ActivationFunctionType.Identity
DR = mybir.MatmulPerfMode.DoubleRow

LOG2E = 1.4426950408889634
A16 = 128.0 * LOG2E
B16 = 127.0 * 128.0 - 4.0

SPLIT_A = 0.525             # fraction of exp pair-tiles on ACT
AVLAG = 6                   # AV trails exp by this many pair-steps


def build():
    nc = bacc.Bacc("TRN2", target_bir_lowering=False, debug=False)

    x8_d = nc.dram_tensor("x8", [128, 2 * N], FP8, kind="ExternalInput")
    xq_d = nc.dram_tensor("xq", [128, 2 * NH], F32, kind="ExternalInput")
    wp8_d = nc.dram_tensor("wp8", [128, 768], FP8, kind="ExternalInput")
    wz_d = nc.dram_tensor("wz", [128, 256], BF16, kind="ExternalInput")
    idn_d = nc.dram_tensor("idn", [128, 128], BF16, kind="ExternalInput")
    bqk_d = nc.dram_tensor("bqk", [128, 2], F32, kind="ExternalInput")
    out_d = nc.dram_tensor("out", [2, 128, NH], F32, kind="ExternalOutput")

    with tile.TileContext(nc) as tc, ExitStack() as ctx:
        const = ctx.enter_context(tc.tile_pool(name="const", bufs=1))
        sb = ctx.enter_context(tc.tile_pool(name="sb", bufs=1))
        exa = ctx.enter_context(tc.tile_pool(name="exa", bufs=8))
        exd = ctx.enter_context(tc.tile_pool(name="exd", bufs=8))
        ynp = ctx.enter_context(tc.tile_pool(name="ynp", bufs=2))
        ytp = ctx.enter_context(tc.tile_pool(name="ytp", bufs=2))
        rcpp = ctx.enter_context(tc.tile_pool(name="rcpp", bufs=2))
        otp = ctx.enter_context(tc.tile_pool(name="otp", bufs=2))

        # ---- DMA preamble (issue order = need order) ----
        wp8_t = const.tile([128, 768], FP8, tag="wp8", name="wp8_t")
        nc.sync.dma_start(wp8_t[:], wp8_d.ap())
        bqk_t = const.tile([128, 2], F32, tag="bqk", name="bqk_t")
        nc.sync.dma_start(bqk_t[:], bqk_d.ap())
        # x8 is plane-major [2, N]; load both planes of each 512-col chunk
        # together so projection chunk j has its full K=256 early
        x8_t = const.tile([128, 2 * N], FP8, tag="x8", name="x8_t")
        for c0 in range(0, N, 512):
            nc.sync.dma_start(x8_t[:, c0:c0 + 512], x8_d.ap()[:, c0:c0 + 512])
            nc.sync.dma_start(x8_t[:, N + c0:N + c0 + 512],
                              x8_d.ap()[:, N + c0:N + c0 + 512])
        wz_t = const.tile([128, 256], BF16, tag="wz", name="wz_t")
        nc.sync.dma_start(wz_t[:], wz_d.ap())
        idn_t = const.tile([128, 128], BF16, tag="idn", name="idn_t")
        nc.sync.dma_start(idn_t[:], idn_d.ap())
        xq_t = const.tile([128, 2 * NH], F32, tag="xq", name="xq_t")
        for c0 in range(0, 2 * NH, 2048):
            nc.sync.dma_start(xq_t[:, c0:c0 + 2048], xq_d.ap()[:, c0:c0 + 2048])

        x8v = x8_t[:].rearrange("p (two n) -> p two n", two=2)       # [128,2,N]
        wq8 = wp8_t[:, 0:256].rearrange("p (two m) -> p two m", two=2)
        wk8 = wp8_t[:, 256:512].rearrange("p (two m) -> p two m", two=2)
        wg8 = wp8_t[:, 512:768].rearrange("p (two m) -> p two m", two=2)

        # ---- persistent SBUF ----
        qT8 = sb.tile([128, 2 * NH], FP8, tag="qT8", name="qT8")     # (2,NH)
        kT8 = sb.tile([128, 2 * N], FP8, tag="kT8", name="kT8")      # (2,N)
        v2 = sb.tile([128, MT * 132], BF16, tag="v2", name="v2")

        # prefetch the ACT Exp table: a no-dep dummy activation up front so
        # the 1283ns LoadActFuncSet runs during the DMA preamble, not on the
        # first fold's critical path
        dummy = sb.tile([128, 1], F32, tag="dummy", name="dummy")
        nc.gpsimd.memset(dummy[:], 0.0)
        nc.scalar.activation(dummy[:], dummy[:], Exp)

        # zero planes / ones col (Pool engine, ordered by tile deps)
        nc.gpsimd.memset(kT8[:, N:N + 512], 0.0)          # covers mt 0..3
        nc.gpsimd.memset(qT8[:, NH:NH + CH], 0.0)         # ch0 plane-1
        v2ones = v2[:].rearrange("p (c w) -> p c w", w=33)[:, :, 32:33]
        nc.gpsimd.memset(v2ones, 1.0)
        nc.gpsimd.memset(kT8[:, N + 512:2 * N], 0.0)
        nc.gpsimd.memset(qT8[:, NH + CH:2 * NH], 0.0)

        with tc.tile_pool(name="sc", bufs=3, space="PSUM") as sc, \
             tc.tile_pool(name="av", bufs=1, space="PSUM") as av:

            # ---- projection emitters ----
            def proj_q(j):
                ps = sc.tile([128, 1024], F32, tag="s", name="qps")
                nc.tensor.matmul(ps[:, 0:CH], wq8,
                                 x8v[:, :, j * CH:(j + 1) * CH],
                                 start=True, stop=True, perf_mode=DR)
                with nc.allow_low_precision("q fold fp8"):
                    nc.scalar.activation(qT8[:, j * CH:(j + 1) * CH],
                                         ps[:, 0:CH], Ident,
                                         bias=bqk_t[:, 0:1])

            def proj_k(j):
                ps = sc.tile([128, 1024], F32, tag="s", name="kps")
                nc.tensor.matmul(ps[:, 0:CH], wk8,
                                 x8v[:, :, j * CH:(j + 1) * CH],
                                 start=True, stop=True, perf_mode=DR)
                with nc.allow_low_precision("k fold fp8"):
                    nc.scalar.activation(kT8[:, j * CH:(j + 1) * CH],
                                         ps[:, 0:CH], Ident,
                                         bias=bqk_t[:, 1:2])

            def proj_v(m2):
                # two key tiles 2*m2, 2*m2+1 -> v2 cols
                ps = sc.tile([128, 1024], F32, tag="s", name="vps")
                for i in range(2):
                    m = 2 * m2 + i
                    nc.tensor.matmul(ps[:, i * 128:i * 128 + 128],
                                     x8v[:, :, m * 128:(m + 1) * 128], wg8,
                                     start=True, stop=True, perf_mode=DR)
                dst = v2[:, m2 * 264:(m2 + 1) * 264] \
                    .rearrange("p (c w) -> p c w", w=33)[:, :, 0:32]
                src = ps[:, 0:256].rearrange("p (c w) -> p c w", w=32)
                with nc.allow_low_precision("v2 bf16"):
                    nc.scalar.copy(dst, src)

            qv = qT8[:].rearrange("p (two n) -> p two n", two=2)
            kv = kT8[:].rearrange("p (two n) -> p two n", two=2)

            # ---- AV + epilogue ----
            st_chains = {}

            def do_av(ch, mt, pr, ex, exbf):
                if mt == 0 and pr == 0:
                    st_chains[ch] = av.tile([128, 1024], F32, tag="av",
                                            name=f"chains{ch}")
                chains = st_chains[ch]
                for i in range(2):
                    h = 2 * pr + i
                    for qt in range(4):
                        c = 4 * h + qt
                        nc.tensor.matmul(
                            chains[:, 64 * c:64 * c + 33],
                            exbf[:, i * CH + qt * 128:i * CH + qt * 128 + 128],
                            v2[:, mt * 132 + h * 33:mt * 132 + h * 33 + 33],
                            start=(mt == 0 and c % 8 == 0), stop=(mt == MT - 1),
                            skip_group_check=True)

            def epilogue(ch):
                chains = st_chains.pop(ch)
                cv = chains[:].rearrange("p (c w) -> p c w", w=64)
                rcp = rcpp.tile([128, 16], F32, tag="r", name="rcp")
                for b_ in range(2):
                    nc.vector.reciprocal(
                        rcp[:, 8 * b_:8 * b_ + 8]
                        .rearrange("p (c u) -> p c u", u=1),
                        cv[:, 8 * b_:8 * b_ + 8, 32:33])
                yn = ynp.tile([128, 512], BF16, tag="yn", name="yn")
                ynv = yn[:].rearrange("p (q w) -> p q w", w=128)
                epi = sc.tile([128, 1024], F32, tag="s", name="epi")
                ytv = epi[:, 0:256].bitcast(BF16)       # [128, 512] bf16
                for qt in range(4):
                    # normalize this q-tile, then its transposes -- PE
                    # pipelines behind DVE and chains free after qt 3
                    with nc.allow_low_precision("normalize bf16"):
                        nc.vector.tensor_tensor(
                            ynv[:, qt, :].rearrange("p (c w) -> p c w", w=32),
                            cv[:, qt::4, 0:32],
                            rcp[:, qt:16:4].rearrange("p (c u) -> p c u", u=1)
                            .broadcast_to([128, 4, 32]),
                            op=MULT)
                    for h in range(4):
                        nc.tensor.matmul(
                            ytv[32 * h:32 * h + 32, 128 * qt:128 * qt + 128],
                            yn[:, 128 * qt + 32 * h:128 * qt + 32 * h + 32],
                            idn_t[:], is_transpose=True,
                            start=False, stop=True, skip_group_check=True,
                            tile_position=(0, 32 * h))
                yT = ytp.tile([128, 512], BF16, tag="yT", name="yT")
                nc.scalar.copy(yT[:], ytv[:])
                zt = av.tile([128, 1024], F32, tag="av", name=f"zt{ch}")
                nc.tensor.matmul(epi[:, 512:1024], wz_t[:, 0:128], yT[:],
                                 start=True, stop=True)
                nc.tensor.matmul(zt[:, 0:512], wz_t[:, 128:256], yT[:],
                                 start=True, stop=True)
                for mtz in range(2):
                    zsrc = epi[:, 512:1024] if mtz == 0 else zt[:, 0:512]
                    o = otp.tile([128, 512], F32, tag="o", name="ot")
                    nc.vector.tensor_tensor(
                        o[:], zsrc, xq_t[:, mtz * NH + ch * CH:
                                         mtz * NH + ch * CH + CH], op=ADD)
                    nc.sync.dma_start(out_d.ap()[mtz, :, ch * CH:(ch + 1) * CH],
                                      o[:])

            # ---- main stream ----
            proj_q(0)
            proj_k(0)
            proj_v(0)

            queue = []
            acc = [0.0]

            def pick_act():
                acc[0] += SPLIT_A
                if acc[0] >= 1.0:
                    acc[0] -= 1.0
                    return True
                return False

            def pop_av():
                ch, mt, pr, ex, exbf = queue.pop(0)
                do_av(ch, mt, pr, ex, exbf)
                if mt == MT - 1 and pr == 1:
                    epilogue(ch)

            for ch in range(NCH):
                for mt in range(MT):
                    if ch == 0:
                        if mt % 4 == 1 and mt < 29:
                            proj_k(mt // 4 + 1)
                        if mt in (3, 11, 19) and mt // 8 + 1 < NCH:
                            proj_q(mt // 8 + 1)
                        if mt % 2 == 0 and mt >= 2:
                            proj_v(mt // 2)
                        if mt == 30:
                            proj_v(15)
                    for pr in range(2):
                        s = sc.tile([128, 1024], F32, tag="s", name="sps")
                        for i in range(2):
                            h = 2 * pr + i
                            nc.tensor.matmul(
                                s[:, i * CH:(i + 1) * CH],
                                kv[32 * h:32 * h + 32, :, mt * 128:(mt + 1) * 128],
                                qv[32 * h:32 * h + 32, :, ch * CH:(ch + 1) * CH],
                                start=True, stop=True, perf_mode=DR,
                                tile_position=(32 * h, 0))
                        if pick_act():
                            ex = exa.tile([128, 1024], BF16, tag="e", name="exa")
                            nc.scalar.activation(ex[:], s[:], Exp)
                            exbf = ex[:]
                        else:
                            ex = exd.tile([128, 1024], I16, tag="e", name="exd")
                            with nc.allow_low_precision("schraudolph exp"):
                                nc.vector.tensor_scalar(ex[:], s[:], A16, B16,
                                                        op0=MULT, op1=ADD)
                            exbf = ex[:].bitcast(BF16)
                        queue.append((ch, mt, pr, ex, exbf))
                        if len(queue) > AVLAG:
                            pop_av()
            while queue:
                pop_av()

    nc.compile()
    return nc


_NC = None


def _get_nc():
    global _NC
    if _NC is None:
        _NC = build()
    return _NC


def _in_maps(x, w_theta, b_theta, w_phi, b_phi, w_g, b_g, w_z, b_z,
             bn_gamma, bn_beta, bn_mean, bn_var):
    sqa = np.float32(np.sqrt(ALPHA))
    inv = (np.asarray(bn_gamma, np.float32)
           / np.sqrt(np.asarray(bn_var, np.float32) + EPS))
    shift = ((np.asarray(w_z, np.float32) @ np.asarray(b_g, np.float32)
              + np.asarray(b_z, np.float32)) * inv
             + np.asarray(bn_beta, np.float32)
             - np.asarray(bn_mean, np.float32) * inv)

    def pack_w(w):  # [INTER, C] -> [128, 256] lhsT planes, fp8
        wT = np.asarray(w, np.float32).T           # [C, INTER]
        return np.concatenate([wT[:128], wT[128:]], axis=1)

    wq = pack_w(np.asarray(w_theta, np.float32) * sqa)
    wk = pack_w(np.asarray(w_phi, np.float32) * sqa)
    wg = pack_w(np.asarray(w_g, np.float32))
    wp8 = np.ascontiguousarray(
        np.concatenate([wq, wk, wg], axis=1)).astype(F8)
    wzs = np.ascontiguousarray(
        (np.asarray(w_z, np.float32) * inv[:, None]).T.astype(BF))  # [128,256]
    bqk = np.ascontiguousarray(np.stack(
        [np.asarray(b_theta, np.float32) * sqa,
         np.asarray(b_phi, np.float32) * sqa], axis=1))
    idn = np.eye(128, dtype=np.float32).astype(BF)

    xr = np.asarray(x, np.float32).reshape(B, C, N)
    shared = {"wp8": wp8, "wz": wzs, "bqk": bqk, "idn": idn}
    maps = []
    for core in range(NCORE):
        b_, half = divmod(core, 2)
        n0 = half * NH
        n1 = NH - n0
        xre = np.concatenate([xr[b_][:, n0:n0 + NH], xr[b_][:, n1:n1 + NH]],
                             axis=1)            # own query half first
        xs = xre.astype(F8)
        x8 = np.empty((128, 2 * N), F8)
        x8[:, 0:N] = xs[:128]
        x8[:, N:2 * N] = xs[128:]
        xqr = xre[:, 0:NH] + shift[:, None]
        xqp = np.concatenate([xqr[:128], xqr[128:]], axis=1).astype(np.float32)
        m = dict(shared)
        m["x8"] = np.ascontiguousarray(x8)
        m["xq"] = np.ascontiguousarray(xqp)
        maps.append(m)
    return maps


def kernel(**inputs):
    nc = _get_nc()
    maps = _in_maps(**inputs)
    res = bass_utils.run_bass_kernel_spmd(nc, maps, core_ids=list(range(NCORE)))
    out = np.empty((B, C, N), np.float32)
    for core in range(NCORE):
        b_, half = divmod(core, 2)
        n0 = half * NH
        out[b_][:, n0:n0 + NH] = res.results[core]["out"].reshape(C, NH)
    return out.reshape(B, C, H, W)
